# revision 1
# baseline (speedup 1.0000x reference)
"""Expert-parallel sparse GLU (MoE) kernel for 8 TRN2 NeuronCores.

Problem: x[16384,1024] tokens pre-sorted by expert, 8 experts with equal
capacity 2048; per expert e:
    out_e = (gelu(x_e @ w1[e].T) * (x_e @ v1[e].T)) @ w2[e]

Sharding: expert parallelism — core e computes expert e on its 2048-token
slice. Zero inter-core communication.

Per-core schedule (all fp32 storage, float32r matmuls = full PE rate):
  - xT [H=1024, cap=2048] resident in SBUF as [128, 8, 2048]
  - two c-blocks of 1024 tokens; per block:
      Phase A: for each f-tile (128 of F=2048): x1T/x2T = w1T/v1T-tile.T @ xT
               accumulated over H in PSUM; GLU (ACT gelu + DVE mul) into
               hT [128, 16, 1024] SBUF
      Phase B: out[c,h'] accumulated over F in PSUM: lhsT = hT f-tiles,
               rhs = streamed w2 tiles [128, 512]
"""

import numpy as np

T, H, F, E = 16384, 1024, 2048, 8
CAP = T // E  # 2048 tokens per expert/core
P = 128
KO = H // P            # 8 h-subtiles
FO = F // P            # 16 f-tiles
NBLK = 2               # c-blocks
CBLK = CAP // NBLK     # 1024
NQ = CBLK // 512       # 2 q-chunks of 512 per block
NCS = CBLK // P        # 8 c-subtiles per block
NH2 = H // 512         # 2 output column halves

_CACHE = {}


def _build_nc(act="Gelu", reps=1, probe_scale=False, split_tail=False,
              q_outer_blk0=False, xt_cmajor=False):
    import concourse.tile as tile
    from concourse import bacc
    import concourse.mybir as mybir

    f32 = mybir.dt.float32
    f32r = mybir.dt.float32r
    Gelu = getattr(mybir.ActivationFunctionType, act)

    nc = bacc.Bacc("TRN2", target_bir_lowering=False, debug=False, num_devices=E)

    xt = nc.dram_tensor("xt", [H, CAP], f32r, kind="ExternalInput").ap()
    w1t = nc.dram_tensor("w1t", [H, F], f32r, kind="ExternalInput").ap()
    v1t = nc.dram_tensor("v1t", [H, F], f32r, kind="ExternalInput").ap()
    w2 = nc.dram_tensor("w2", [F, H], f32r, kind="ExternalInput").ap()
    out = nc.dram_tensor("out", [CAP, H], f32, kind="ExternalOutput").ap()

    xt3 = xt.rearrange("(ko p) c -> p ko c", p=P)    # [128, 8, 2048]
    w1t3 = w1t.rearrange("(ko p) f -> p ko f", p=P)  # [128, 8, 2048]
    v1t3 = v1t.rearrange("(ko p) f -> p ko f", p=P)

    with tile.TileContext(nc) as tc:
        with (
            tc.tile_pool(name="htp", bufs=1) as htp,
            tc.tile_pool(name="wap", bufs=3) as wap,
            tc.tile_pool(name="wbp", bufs=6) as wbp,
            tc.tile_pool(name="tmpp", bufs=3) as tmpp,
            tc.tile_pool(name="obp", bufs=6) as obp,
            tc.tile_pool(name="psp", bufs=8, space="PSUM") as psp,
        ):
          for _rep in range(reps):  # reps>1 only for steady-state timing
           with tc.tile_pool(name="xtp", bufs=1) as xtp:
            def load_w(fo):
                fsl = slice(fo * P, (fo + 1) * P)
                w1s = wap.tile([P, KO, P], f32r, tag="w1s", name="w1s")
                nc.sync.dma_start(w1s[:], w1t3[:, :, fsl])
                v1s = wap.tile([P, KO, P], f32r, tag="v1s", name="v1s")
                nc.sync.dma_start(v1s[:], v1t3[:, :, fsl])
                return w1s, v1s

            # startup order: fo0 weights, xt[ko0], fo1 weights, xt[ko1..7]
            # — the first accumulation group's MMs start after ~4 MB instead
            # of waiting behind the whole 8 MB xT load
            wpre = {}
            xts = xtp.tile([P, KO, CAP], f32r, name="xts")
            # HAM warm-up: the PE idles ~3-5 us waiting for the first DMAs
            # and would then run its first ~3.4 us of real matmuls at the
            # cold 1.2 GHz clock. Burn that idle window on dummy matmuls
            # over a zeroed tile so the activity monitor un-throttles the
            # clock before real work arrives.
            if _rep == 0:
                wz0 = tmpp.tile([P, 128], f32, name="wz0", tag="wz0", bufs=1)
                nc.vector.memset(wz0[:], 0.0)
                wz = tmpp.tile([P, 128], f32r, name="wz", tag="wz", bufs=1)
                nc.vector.tensor_copy(wz[:], wz0[:])
                # ~32 cold matmuls ≈ 3.4 us — one full HAM activity window,
                # so the PE clock is un-throttled when real work arrives;
                # still shorter than the first DMA wait, so it costs nothing
                for wi in range(32):
                    pz = psp.tile([P, 128], f32, tag="ps", name="pz")
                    nc.tensor.matmul(pz[:], wz[:], wz[:],
                                     start=True, stop=True)
            # first f-tile's weights in per-ko pieces: the first real matmul
            # is gated by just 64 KB of w1 + 256 KB of xT
            w1s0 = wap.tile([P, KO, P], f32r, tag="w1s", name="w1s")
            nc.sync.dma_start(w1s0[:, 0, :], w1t3[:, 0, 0:P])
            if not xt_cmajor:
                nc.sync.dma_start(xts[:, 0, 0:512], xt3[:, 0, 0:512])
            v1s0 = wap.tile([P, KO, P], f32r, tag="v1s", name="v1s")
            nc.sync.dma_start(v1s0[:, 0, :], v1t3[:, 0, 0:P])
            if not xt_cmajor:
                nc.sync.dma_start(xts[:, 0, 512:CBLK], xt3[:, 0, 512:CBLK])
            nc.sync.dma_start(w1s0[:, 1:KO, :], w1t3[:, 1:KO, 0:P])
            nc.sync.dma_start(v1s0[:, 1:KO, :], v1t3[:, 1:KO, 0:P])
            wpre[0] = (w1s0, v1s0)
            if xt_cmajor:
                for qq in range(CAP // 512):
                    for ko in range(KO):
                        csl = slice(qq * 512, (qq + 1) * 512)
                        nc.sync.dma_start(xts[:, ko, csl], xt3[:, ko, csl])
                wpre[1] = load_w(1)
            else:
                # block 0 only reads columns 0:CBLK — load those halves
                # first so the ramp is gated by 4 MB, not 8 MB (ko0's block-0
                # columns were already queued above)
                wpre[1] = load_w(1)
                for ko in range(1, KO):
                    nc.sync.dma_start(xts[:, ko, 0:CBLK], xt3[:, ko, 0:CBLK])

            # hT for one c-block: [f%128, f//128, c within block]
            hts = htp.tile([P, FO, CBLK], f32r, name="hts")

            def emit_a(c0, fo, qs, w1s, v1s):
                x1p = {q: psp.tile([P, 512], f32, tag="ps", name="x1p")
                       for q in qs}
                x2p = {q: psp.tile([P, 512], f32, tag="ps", name="x2p")
                       for q in qs}
                for ko in range(KO):
                    st = dict(start=(ko == 0), stop=(ko == KO - 1))
                    w1k = w1s[:, ko, :]
                    v1k = v1s[:, ko, :]
                    for q in qs:
                        xk = xts[:, ko, c0 + q * 512: c0 + (q + 1) * 512]
                        nc.tensor.matmul(x1p[q][:], w1k, xk, **st)
                    for q in qs:
                        xk = xts[:, ko, c0 + q * 512: c0 + (q + 1) * 512]
                        nc.tensor.matmul(x2p[q][:], v1k, xk, **st)
                for q in qs:
                    gtmp = tmpp.tile([P, 512], f32, name="gtmp")
                    nc.scalar.activation(gtmp[:], x1p[q][:], Gelu)
                    nc.vector.tensor_mul(
                        hts[:, fo, q * 512:(q + 1) * 512], gtmp[:], x2p[q][:]
                    )

            def emit_b_pass(c0, h2, cs_list, cached=None, preload=None,
                            split_copy=False):
                hsl = slice(h2 * 512, (h2 + 1) * 512)
                op = {cs: psp.tile([P, 512], f32, tag="ps", name=f"op{cs}")
                      for cs in cs_list}
                for fo in range(FO):
                    if cached is not None:
                        w2r = cached[:, fo, :]
                    else:
                        w2s = wbp.tile([P, 512], f32r, tag="w2s", name="w2s")
                        nc.sync.dma_start(w2s[:], w2[fo * P:(fo + 1) * P, hsl])
                        w2r = w2s[:]
                    if preload is not None:
                        # ride-along DMA filling the w2 cache for the NEXT
                        # (final) half-passes
                        nc.sync.dma_start(preload[:, fo, :],
                                          w2[fo * P:(fo + 1) * P, 512:1024])
                    st = dict(start=(fo == 0), stop=(fo == FO - 1))
                    for cs in cs_list:
                        hk = hts[:, fo, cs * P:(cs + 1) * P]
                        nc.tensor.matmul(op[cs][:], hk, w2r, **st)
                for ci, cs in enumerate(cs_list):
                    ob = obp.tile([P, 512], f32, name="ob")
                    if probe_scale and _rep == reps - 1:
                        nc.scalar.mul(ob[:], op[cs][:], 2.0)
                    elif split_copy and ci % 2 == 1:
                        nc.scalar.copy(ob[:], op[cs][:])
                    else:
                        nc.vector.tensor_copy(ob[:], op[cs][:])
                    nc.sync.dma_start(
                        out[c0 + cs * P: c0 + (cs + 1) * P, hsl], ob[:])

            for blk in range(NBLK):
                c0 = blk * CBLK
                # ---------------- Phase A: x1T/x2T + GLU -> hT ----------
                for fo in range(FO):
                    if blk == 0 and fo in wpre:
                        w1s, v1s = wpre[fo]
                    else:
                        w1s, v1s = load_w(fo)
                    if blk == 0 and 2 <= fo < 2 + KO:
                        ko = fo - 2
                        nc.sync.dma_start(xts[:, ko, CBLK:CAP],
                                          xt3[:, ko, CBLK:CAP])
                    emit_a(c0, fo, list(range(NQ)), w1s, v1s)
                # ---------------- Phase B (block 0 only here) -----------
                if blk == 0:
                    for h2 in range(NH2):
                        emit_b_pass(c0, h2, list(range(NCS)))
           # xts dead from here on — release its 64 KB/partition and cache
           # all of w2's h2=1 half there so the final half-passes have no
           # DMA dependence and drain early
           with tc.tile_pool(name="w2c", bufs=1) as w2c:
                w2cache = w2c.tile([P, FO, 512], f32r, name="w2cache")
                emit_b_pass(CBLK, 0, list(range(NCS)), preload=w2cache)
                emit_b_pass(CBLK, 1, [0, 1, 2, 3], cached=w2cache,
                            split_copy=True)
                emit_b_pass(CBLK, 1, [4, 5, 6, 7], cached=w2cache,
                            split_copy=True)
    nc.finalize()  # bacc register allocation + codegen passes
    return nc


def _get_nc():
    if "nc" not in _CACHE:
        _CACHE["nc"] = _build_nc()
    return _CACHE["nc"]


def kernel(x, w1, v1, w2, expert_ids):
    """Full inputs in, full output out. expert_ids is ignored: tokens are
    pre-sorted with equal capacity T//E (the reference ignores it too)."""
    from concourse.bass_utils import run_bass_kernel_spmd

    nc = _get_nc()

    x = np.asarray(x, dtype=np.float32)
    w1 = np.asarray(w1, dtype=np.float32)
    v1 = np.asarray(v1, dtype=np.float32)
    w2 = np.asarray(w2, dtype=np.float32)

    in_maps = []
    for e in range(E):
        xs = x[e * CAP:(e + 1) * CAP]  # [cap, H]
        in_maps.append({
            "xt": np.ascontiguousarray(xs.T),           # [H, cap]
            "w1t": np.ascontiguousarray(w1[e].T),       # [H, F]
            "v1t": np.ascontiguousarray(v1[e].T),       # [H, F]
            "w2": np.ascontiguousarray(w2[e]),          # [F, H]
        })

    try:
        res = run_bass_kernel_spmd(nc, in_maps, core_ids=list(range(E)))
    except Exception:
        # transient NRT/device errors (e.g. a core left wedged by an earlier
        # process) usually clear on retry
        res = run_bass_kernel_spmd(nc, in_maps, core_ids=list(range(E)))
    outs = [res.results[e]["out"] for e in range(E)]
    return np.concatenate(outs, axis=0).astype(np.float32)



# revision 2
# speedup vs baseline: 1.0309x; 1.0309x over previous
"""Expert-parallel sparse GLU (MoE) kernel for 8 TRN2 NeuronCores.

Problem: x[16384,1024] tokens pre-sorted by expert, 8 experts with equal
capacity 2048; per expert e:
    out_e = (gelu(x_e @ w1[e].T) * (x_e @ v1[e].T)) @ w2[e]

Sharding: expert parallelism — core e computes expert e on its 2048-token
slice. Zero inter-core communication.

Precision: all inputs are cast to bf16 on the host (PE rate is identical
to fp32r, but DMA traffic and SBUF footprint halve; output rel err ~3e-3
vs the 2e-2 gate). PSUM accumulation stays fp32; out is fp32.

Per-core schedule:
  - xT [H=1024, cap=2048] resident in SBUF as [128, 8, 2048] bf16,
    double-buffered across reps so rep N+1's 4 MB load overlaps rep N's
    Phase B (kills the rep-boundary PE stall).
  - two c-blocks of 1024 tokens; per block:
      Phase A: for each f-tile (128 of F=2048): x1T/x2T = w1T/v1T-tile.T @ xT
               accumulated over H in PSUM; GLU (ACT gelu + DVE mul) into
               hT [128, 16, 1024] bf16 SBUF
      Phase B: out[c,h'] accumulated over F in PSUM: lhsT = hT f-tiles,
               rhs = streamed w2 tiles [128, 512] bf16
  - block 1's h2=1 Phase B half runs from a ride-along-filled SBUF w2
    cache so the final passes have no DMA dependence.
"""

import numpy as np

T, H, F, E = 16384, 1024, 2048, 8
CAP = T // E  # 2048 tokens per expert/core
P = 128
KO = H // P            # 8 h-subtiles
FO = F // P            # 16 f-tiles
NBLK = 2               # c-blocks
CBLK = CAP // NBLK     # 1024
NQ = CBLK // 512       # 2 q-chunks of 512 per block
NCS = CBLK // P        # 8 c-subtiles per block
NH2 = H // 512         # 2 output column halves

_CACHE = {}


def _build_nc(act="Gelu", reps=1, probe_scale=False):
    import concourse.tile as tile
    from concourse import bacc
    import concourse.mybir as mybir

    f32 = mybir.dt.float32
    bf16 = mybir.dt.bfloat16
    Gelu = getattr(mybir.ActivationFunctionType, act)

    nc = bacc.Bacc("TRN2", target_bir_lowering=False, debug=False, num_devices=E)

    xt = nc.dram_tensor("xt", [H, CAP], bf16, kind="ExternalInput").ap()
    w1t = nc.dram_tensor("w1t", [H, F], bf16, kind="ExternalInput").ap()
    v1t = nc.dram_tensor("v1t", [H, F], bf16, kind="ExternalInput").ap()
    w2 = nc.dram_tensor("w2", [F, H], bf16, kind="ExternalInput").ap()
    out = nc.dram_tensor("out", [CAP, H], f32, kind="ExternalOutput").ap()

    xt3 = xt.rearrange("(ko p) c -> p ko c", p=P)    # [128, 8, 2048]
    w1t3 = w1t.rearrange("(ko p) f -> p ko f", p=P)  # [128, 8, 2048]
    v1t3 = v1t.rearrange("(ko p) f -> p ko f", p=P)

    with tile.TileContext(nc) as tc:
        with (
            tc.tile_pool(name="htp", bufs=1) as htp,
            tc.tile_pool(name="xtp", bufs=2) as xtp,
            tc.tile_pool(name="wap", bufs=3) as wap,
            tc.tile_pool(name="wbp", bufs=6) as wbp,
            tc.tile_pool(name="tmpp", bufs=3) as tmpp,
            tc.tile_pool(name="obp", bufs=6) as obp,
            tc.tile_pool(name="w2c", bufs=2) as w2c,
            tc.tile_pool(name="psp", bufs=8, space="PSUM") as psp,
        ):
          for _rep in range(reps):  # reps>1 only for steady-state timing
            def load_w(fo):
                fsl = slice(fo * P, (fo + 1) * P)
                w1s = wap.tile([P, KO, P], bf16, tag="w1s", name="w1s")
                nc.sync.dma_start(w1s[:], w1t3[:, :, fsl])
                v1s = wap.tile([P, KO, P], bf16, tag="v1s", name="v1s")
                nc.sync.dma_start(v1s[:], v1t3[:, :, fsl])
                return w1s, v1s

            # startup order: fo0 weights, xt[ko0], fo1 weights, xt[ko1..7]
            # — the first accumulation group's MMs start after ~2 MB instead
            # of waiting behind the whole 4 MB xT load
            wpre = {}
            xts = xtp.tile([P, KO, CAP], bf16, tag="xts", name="xts")
            # HAM warm-up: the PE idles ~3-5 us waiting for the first DMAs
            # and would then run its first ~3.4 us of real matmuls at the
            # cold 1.2 GHz clock. Burn that idle window on dummy matmuls
            # over a zeroed tile so the activity monitor un-throttles the
            # clock before real work arrives.
            if _rep == 0:
                wz = tmpp.tile([P, 128], bf16, name="wz", tag="wz", bufs=1)
                nc.vector.memset(wz[:], 0.0)
                # ~32 cold matmuls ≈ 3.4 us — one full HAM activity window,
                # so the PE clock is un-throttled when real work arrives;
                # still shorter than the first DMA wait, so it costs nothing
                for wi in range(32):
                    pz = psp.tile([P, 128], f32, tag="ps", name="pz")
                    nc.tensor.matmul(pz[:], wz[:], wz[:],
                                     start=True, stop=True)
            # first f-tile's weights in per-ko pieces: the first real matmul
            # is gated by just 32 KB of w1 + 128 KB of xT
            w1s0 = wap.tile([P, KO, P], bf16, tag="w1s", name="w1s")
            nc.sync.dma_start(w1s0[:, 0, :], w1t3[:, 0, 0:P])
            nc.sync.dma_start(xts[:, 0, 0:512], xt3[:, 0, 0:512])
            v1s0 = wap.tile([P, KO, P], bf16, tag="v1s", name="v1s")
            nc.sync.dma_start(v1s0[:, 0, :], v1t3[:, 0, 0:P])
            nc.sync.dma_start(xts[:, 0, 512:CBLK], xt3[:, 0, 512:CBLK])
            nc.sync.dma_start(w1s0[:, 1:KO, :], w1t3[:, 1:KO, 0:P])
            nc.sync.dma_start(v1s0[:, 1:KO, :], v1t3[:, 1:KO, 0:P])
            wpre[0] = (w1s0, v1s0)
            # block 0 only reads columns 0:CBLK — load those halves
            # first so the ramp is gated by 2 MB, not 4 MB (ko0's block-0
            # columns were already queued above)
            wpre[1] = load_w(1)
            for ko in range(1, KO):
                nc.sync.dma_start(xts[:, ko, 0:CBLK], xt3[:, ko, 0:CBLK])

            # hT for one c-block: [f%128, f//128, c within block]
            hts = htp.tile([P, FO, CBLK], bf16, tag="hts", name="hts")

            def emit_a(c0, fo, qs, w1s, v1s):
                x1p = {q: psp.tile([P, 512], f32, tag="ps", name="x1p")
                       for q in qs}
                x2p = {q: psp.tile([P, 512], f32, tag="ps", name="x2p")
                       for q in qs}
                for ko in range(KO):
                    st = dict(start=(ko == 0), stop=(ko == KO - 1))
                    w1k = w1s[:, ko, :]
                    v1k = v1s[:, ko, :]
                    for q in qs:
                        xk = xts[:, ko, c0 + q * 512: c0 + (q + 1) * 512]
                        nc.tensor.matmul(x1p[q][:], w1k, xk, **st)
                    for q in qs:
                        xk = xts[:, ko, c0 + q * 512: c0 + (q + 1) * 512]
                        nc.tensor.matmul(x2p[q][:], v1k, xk, **st)
                for q in qs:
                    gtmp = tmpp.tile([P, 512], f32, name="gtmp")
                    nc.scalar.activation(gtmp[:], x1p[q][:], Gelu)
                    nc.vector.tensor_mul(
                        hts[:, fo, q * 512:(q + 1) * 512], gtmp[:], x2p[q][:]
                    )

            def emit_b_pass(c0, h2, cs_list, cached=None, preload=None,
                            split_copy=False):
                hsl = slice(h2 * 512, (h2 + 1) * 512)
                op = {cs: psp.tile([P, 512], f32, tag="ps", name=f"op{cs}")
                      for cs in cs_list}
                for fo in range(FO):
                    if cached is not None:
                        w2r = cached[:, fo, :]
                    else:
                        w2s = wbp.tile([P, 512], bf16, tag="w2s", name="w2s")
                        nc.sync.dma_start(w2s[:], w2[fo * P:(fo + 1) * P, hsl])
                        w2r = w2s[:]
                    if preload is not None:
                        # ride-along DMA filling the w2 cache for the NEXT
                        # (final) half-passes
                        nc.sync.dma_start(preload[:, fo, :],
                                          w2[fo * P:(fo + 1) * P, 512:1024])
                    st = dict(start=(fo == 0), stop=(fo == FO - 1))
                    for cs in cs_list:
                        hk = hts[:, fo, cs * P:(cs + 1) * P]
                        nc.tensor.matmul(op[cs][:], hk, w2r, **st)
                for ci, cs in enumerate(cs_list):
                    ob = obp.tile([P, 512], f32, name="ob")
                    if probe_scale and _rep == reps - 1:
                        nc.scalar.mul(ob[:], op[cs][:], 2.0)
                    elif split_copy and ci % 2 == 1:
                        nc.scalar.copy(ob[:], op[cs][:])
                    else:
                        nc.vector.tensor_copy(ob[:], op[cs][:])
                    nc.sync.dma_start(
                        out[c0 + cs * P: c0 + (cs + 1) * P, hsl], ob[:])

            for blk in range(NBLK):
                c0 = blk * CBLK
                # ---------------- Phase A: x1T/x2T + GLU -> hT ----------
                for fo in range(FO):
                    if blk == 0 and fo in wpre:
                        w1s, v1s = wpre[fo]
                    else:
                        w1s, v1s = load_w(fo)
                    if blk == 0 and 2 <= fo < 2 + KO:
                        ko = fo - 2
                        nc.sync.dma_start(xts[:, ko, CBLK:CAP],
                                          xt3[:, ko, CBLK:CAP])
                    emit_a(c0, fo, list(range(NQ)), w1s, v1s)
                # ---------------- Phase B (block 0 only here) -----------
                if blk == 0:
                    for h2 in range(NH2):
                        emit_b_pass(c0, h2, list(range(NCS)))
            # block 1 Phase B: cache all of w2's h2=1 half in SBUF via a
            # ride-along DMA so the final half-passes have no DMA
            # dependence and drain early
            w2cache = w2c.tile([P, FO, 512], bf16, tag="w2cache",
                               name="w2cache")
            emit_b_pass(CBLK, 0, list(range(NCS)), preload=w2cache)
            emit_b_pass(CBLK, 1, [0, 1, 2, 3], cached=w2cache,
                        split_copy=True)
            emit_b_pass(CBLK, 1, [4, 5, 6, 7], cached=w2cache,
                        split_copy=True)
    nc.finalize()  # bacc register allocation + codegen passes
    return nc


def _get_nc():
    if "nc" not in _CACHE:
        _CACHE["nc"] = _build_nc()
    return _CACHE["nc"]


def make_in_maps(x, w1, v1, w2):
    """Host-side shard + bf16 cast (one source of truth for test.py too)."""
    import ml_dtypes

    bf = ml_dtypes.bfloat16
    x = np.asarray(x, dtype=np.float32)
    in_maps = []
    for e in range(E):
        xs = x[e * CAP:(e + 1) * CAP]  # [cap, H]
        in_maps.append({
            "xt": np.ascontiguousarray(xs.T).astype(bf),          # [H, cap]
            "w1t": np.ascontiguousarray(
                np.asarray(w1[e], np.float32).T).astype(bf),      # [H, F]
            "v1t": np.ascontiguousarray(
                np.asarray(v1[e], np.float32).T).astype(bf),      # [H, F]
            "w2": np.asarray(w2[e], np.float32).astype(bf),       # [F, H]
        })
    return in_maps


def kernel(x, w1, v1, w2, expert_ids):
    """Full inputs in, full output out. expert_ids is ignored: tokens are
    pre-sorted with equal capacity T//E (the reference ignores it too)."""
    from concourse.bass_utils import run_bass_kernel_spmd

    nc = _get_nc()
    in_maps = make_in_maps(x, w1, v1, w2)

    try:
        res = run_bass_kernel_spmd(nc, in_maps, core_ids=list(range(E)))
    except Exception:
        # transient NRT/device errors (e.g. a core left wedged by an earlier
        # process) usually clear on retry
        res = run_bass_kernel_spmd(nc, in_maps, core_ids=list(range(E)))
    outs = [res.results[e]["out"] for e in range(E)]
    return np.concatenate(outs, axis=0).astype(np.float32)


# revision 3
# speedup vs baseline: 1.0574x; 1.0258x over previous
"""Expert-parallel sparse GLU (MoE) kernel for 8 TRN2 NeuronCores — v2.

Per expert e (core e): out_e = (gelu(x_e @ w1[e].T) * (x_e @ v1[e].T)) @ w2[e]
x_e [2048, 1024]; w1/v1/w2 [2048, 1024]-shaped per expert. Zero comms.

v2 vs v1: fo-outer Phase A (each w1/v1 f-tile loaded ONCE per rep, used by
the full 2048-token capacity), full w2 SBUF-cached (read once), bf16 output.
Per-rep HBM traffic: xT 4 MB + w1/v1 8 MB + w2 4 MB + out 4 MB = 20 MB
(v1: 34 MB, fp32 baseline: 60 MB). All matmuls bf16 (full PE rate, fp32
PSUM); rel err ~4e-3 vs the 2e-2 gate.

Schedule per rep:
  Phase A: for each f-pair j (2 f-tiles of 128): DMA w1/v1 pair tiles;
           x1T/x2T [128f, 4x512c] accumulated over H in PSUM (8 banks);
           ACT gelu + DVE mul -> hts [128, 16fo, 2048c] bf16.
  Phase B: 4 passes of (8 c-subtiles x 512 h-half): pass 1 streams w2
           h2=0 into a [128,16,512] cache (+ ride-along preload of the
           h2=1 half); passes 2-4 read the caches — no DMA dependence,
           so the tail drains early and the next rep's xT load (single
           xts buffer, WAR-released at end of Phase A) overlaps Phase B.
"""

import numpy as np

T, H, F, E = 16384, 1024, 2048, 8
CAP = T // E  # 2048 tokens per expert/core
P = 128
KO = H // P            # 8 h-subtiles
FO = F // P            # 16 f-tiles
NQ = CAP // 512        # 4 q-chunks of 512 tokens
NCS = CAP // P         # 16 c-subtiles
NH2 = H // 512         # 2 output column halves

_CACHE = {}


def _build_nc(act="Gelu", reps=1, probe_scale=False):
    import concourse.tile as tile
    from concourse import bacc
    import concourse.mybir as mybir

    f32 = mybir.dt.float32
    bf16 = mybir.dt.bfloat16
    Gelu = getattr(mybir.ActivationFunctionType, act)

    nc = bacc.Bacc("TRN2", target_bir_lowering=False, debug=False, num_devices=E)

    xt = nc.dram_tensor("xt", [H, CAP], bf16, kind="ExternalInput").ap()
    w1t = nc.dram_tensor("w1t", [H, F], bf16, kind="ExternalInput").ap()
    v1t = nc.dram_tensor("v1t", [H, F], bf16, kind="ExternalInput").ap()
    w2 = nc.dram_tensor("w2", [F, H], bf16, kind="ExternalInput").ap()
    out = nc.dram_tensor("out", [CAP, H], bf16, kind="ExternalOutput").ap()

    xt3 = xt.rearrange("(ko p) c -> p ko c", p=P)    # [128, 8, 2048]
    w1t3 = w1t.rearrange("(ko p) f -> p ko f", p=P)  # [128, 8, 2048]
    v1t3 = v1t.rearrange("(ko p) f -> p ko f", p=P)

    with tile.TileContext(nc) as tc:
        with (
            tc.tile_pool(name="htp", bufs=1) as htp,
            tc.tile_pool(name="xtp", bufs=1) as xtp,
            tc.tile_pool(name="wap", bufs=3) as wap,
            tc.tile_pool(name="tmpp", bufs=3) as tmpp,
            tc.tile_pool(name="obp", bufs=6) as obp,
            tc.tile_pool(name="w2cA", bufs=2) as w2cA,
            tc.tile_pool(name="w2cB", bufs=2) as w2cB,
            tc.tile_pool(name="psp", bufs=8, space="PSUM") as psp,
        ):
          for _rep in range(reps):  # reps>1 only for steady-state timing
            def load_wpair(j):
                fsl = slice(j * 2 * P, (j + 1) * 2 * P)
                w1s = wap.tile([P, KO, 2 * P], bf16, tag="w1s", name="w1s")
                nc.sync.dma_start(w1s[:], w1t3[:, :, fsl])
                v1s = wap.tile([P, KO, 2 * P], bf16, tag="v1s", name="v1s")
                nc.sync.dma_start(v1s[:], v1t3[:, :, fsl])
                return w1s, v1s

            xts = xtp.tile([P, KO, CAP], bf16, tag="xts", name="xts")
            # HAM warm-up (rep 0 only): burn the first-DMA wait on dummy
            # matmuls over a zeroed tile so the PE clock is un-throttled
            # when real work arrives.
            if _rep == 0:
                wz = tmpp.tile([P, 128], bf16, name="wz", tag="wz", bufs=1)
                nc.vector.memset(wz[:], 0.0)
                for wi in range(32):
                    pz = psp.tile([P, 128], f32, tag="ps", name="pz")
                    nc.tensor.matmul(pz[:], wz[:], wz[:],
                                     start=True, stop=True)
            # startup order: pair-0 weights' first ko rows, then xT ko0,
            # then the rest — the first accumulation group is gated by
            # ~160 KB, not the full 4 MB xT load
            w1s0 = wap.tile([P, KO, 2 * P], bf16, tag="w1s", name="w1s")
            nc.sync.dma_start(w1s0[:, 0, :], w1t3[:, 0, 0:2 * P])
            nc.sync.dma_start(xts[:, 0, :], xt3[:, 0, :])
            v1s0 = wap.tile([P, KO, 2 * P], bf16, tag="v1s", name="v1s")
            nc.sync.dma_start(v1s0[:, 0, :], v1t3[:, 0, 0:2 * P])
            nc.sync.dma_start(w1s0[:, 1:KO, :], w1t3[:, 1:KO, 0:2 * P])
            nc.sync.dma_start(v1s0[:, 1:KO, :], v1t3[:, 1:KO, 0:2 * P])
            wpair0 = (w1s0, v1s0)
            for ko in range(1, KO):
                nc.sync.dma_start(xts[:, ko, :], xt3[:, ko, :])

            # hT for the full capacity: [f%128, f//128, c]
            hts = htp.tile([P, FO, CAP], bf16, tag="hts", name="hts")

            def emit_a(fo, w1s, v1s, col):
                x1p = {q: psp.tile([P, 512], f32, tag="ps", name="x1p")
                       for q in range(NQ)}
                x2p = {q: psp.tile([P, 512], f32, tag="ps", name="x2p")
                       for q in range(NQ)}
                csl = slice(col * P, (col + 1) * P)
                for ko in range(KO):
                    st = dict(start=(ko == 0), stop=(ko == KO - 1))
                    w1k = w1s[:, ko, csl]
                    v1k = v1s[:, ko, csl]
                    for q in range(NQ):
                        xk = xts[:, ko, q * 512:(q + 1) * 512]
                        nc.tensor.matmul(x1p[q][:], w1k, xk, **st)
                    for q in range(NQ):
                        xk = xts[:, ko, q * 512:(q + 1) * 512]
                        nc.tensor.matmul(x2p[q][:], v1k, xk, **st)
                for q in range(NQ):
                    gtmp = tmpp.tile([P, 512], f32, name="gtmp")
                    nc.scalar.activation(gtmp[:], x1p[q][:], Gelu)
                    nc.vector.tensor_mul(
                        hts[:, fo, q * 512:(q + 1) * 512], gtmp[:], x2p[q][:]
                    )

            # ---------------- Phase A ----------------
            w2c0 = w2cA.tile([P, FO, 512], bf16, tag="w2c0", name="w2c0")
            w2c1 = w2cB.tile([P, FO, 512], bf16, tag="w2c1", name="w2c1")
            for j in range(FO // 2):
                w1s, v1s = wpair0 if j == 0 else load_wpair(j)
                emit_a(2 * j, w1s, v1s, 0)
                emit_a(2 * j + 1, w1s, v1s, 1)
                if j == 4:
                    # fill both w2 SBUF caches mid-Phase-A: DMA is idle
                    # here, and Phase B then has no DMA dependence at all
                    for fo in range(FO):
                        nc.sync.dma_start(w2c0[:, fo, :],
                                          w2[fo * P:(fo + 1) * P, 0:512])
                        nc.sync.dma_start(w2c1[:, fo, :],
                                          w2[fo * P:(fo + 1) * P, 512:1024])

            # ---------------- Phase B (cs-outer: one PSUM tile
            # accumulates over all fo, copies overlap the next tile) ----
            for h2 in range(NH2):
                hsl = slice(h2 * 512, (h2 + 1) * 512)
                w2tile = w2c0 if h2 == 0 else w2c1
                for cs in range(NCS):
                    op = psp.tile([P, 512], f32, tag="ps", name=f"op{cs}")
                    for fo in range(FO):
                        st = dict(start=(fo == 0), stop=(fo == FO - 1))
                        hk = hts[:, fo, cs * P:(cs + 1) * P]
                        nc.tensor.matmul(op[:], hk, w2tile[:, fo, :], **st)
                    ob = obp.tile([P, 512], bf16, name="ob")
                    if probe_scale and _rep == reps - 1:
                        nc.scalar.mul(ob[:], op[:], 2.0)
                    elif cs % 2 == 1:
                        nc.scalar.copy(ob[:], op[:])
                    else:
                        nc.vector.tensor_copy(ob[:], op[:])
                    nc.sync.dma_start(
                        out[cs * P:(cs + 1) * P, hsl], ob[:])
    nc.finalize()  # bacc register allocation + codegen passes
    return nc


def _get_nc():
    if "nc" not in _CACHE:
        _CACHE["nc"] = _build_nc()
    return _CACHE["nc"]


def make_in_maps(x, w1, v1, w2):
    """Host-side shard + bf16 cast (one source of truth for test.py too)."""
    import ml_dtypes

    bf = ml_dtypes.bfloat16
    x = np.asarray(x, dtype=np.float32)
    in_maps = []
    for e in range(E):
        xs = x[e * CAP:(e + 1) * CAP]  # [cap, H]
        in_maps.append({
            "xt": np.ascontiguousarray(xs.T).astype(bf),          # [H, cap]
            "w1t": np.ascontiguousarray(
                np.asarray(w1[e], np.float32).T).astype(bf),      # [H, F]
            "v1t": np.ascontiguousarray(
                np.asarray(v1[e], np.float32).T).astype(bf),      # [H, F]
            "w2": np.asarray(w2[e], np.float32).astype(bf),       # [F, H]
        })
    return in_maps


def kernel(x, w1, v1, w2, expert_ids):
    """Full inputs in, full output out. expert_ids is ignored: tokens are
    pre-sorted with equal capacity T//E (the reference ignores it too)."""
    from concourse.bass_utils import run_bass_kernel_spmd

    nc = _get_nc()
    in_maps = make_in_maps(x, w1, v1, w2)

    try:
        res = run_bass_kernel_spmd(nc, in_maps, core_ids=list(range(E)))
    except Exception:
        # transient NRT/device errors (e.g. a core left wedged by an earlier
        # process) usually clear on retry
        res = run_bass_kernel_spmd(nc, in_maps, core_ids=list(range(E)))
    outs = [res.results[e]["out"] for e in range(E)]
    return np.concatenate(outs, axis=0).astype(np.float32)
